# revision 12
# baseline (speedup 1.0000x reference)
"""Trainium2 Bass kernel for nn_Attention_28552942584284 (sparse_attention).

Reference computation (per batch b):
    scores  = exp(Q @ K^T) / sqrt(d)          # exp BEFORE scaling
    scores  = where(mask == 0, -1e9, scores)
    p_attn  = softmax(scores, axis=-1)
    out     = (p_attn @ V).sum(axis=q)        # == colsum(p_attn) @ V
    returns (out, p_attn)

Sharding: data-parallel over batch B=16 across 8 cores (2 batches/core).

Per-core device pipeline (scores stay in [q-partition, kv-free] layout):
    qk       = Qt.T @ Kt                      (bf16 matmul, PSUM fp32)
    e1       = Exp(qk)                        (ACT, PSUM->SBUF fp32)
    t        = e1 + mbias                     (DVE; mbias = (mask-1)*2^34, so
                                               masked lanes become ~-2^34)
    p_un, Z  = Exp(t * (1/sqrt(d)))           (ACT with accum_out => rowsum;
                                               masked lanes underflow to 0.0,
                                               matching reference exactly)
    recip    = 1/Z                            (DVE reciprocal)
    p_attn   = p_un * recip                   (DVE tensor_scalar, -> DRAM)
    c        = sum_q p_attn = recip.T @ p_un  (PE matmul, ones trick w/ recip)
    out      = c.T @ V                        (PE matmul after a tiny SBUF
                                               reshape DMA of c)

The softmax max-subtraction is skipped deliberately: unmasked scores lie in
~[0.05, 0.08] and masked scores are -1e9, so exp() is numerically safe and
exp(-1e9) == 0.0 in fp32 exactly as in the reference.
"""

import math

import numpy as np
import ml_dtypes

import concourse.bass as bass
import concourse.bacc as bacc
import concourse.mybir as mybir
import concourse.tile as tile
from concourse.bass import ts
from concourse.bass_utils import run_bass_kernel_spmd

# Problem constants (hardcoded; kernel.py must be self-contained).
B, SQ, SKV, D = 16, 1024, 1024, 256
NCORES = 8
NB = B // NCORES  # batches per core
P = 128
LAM = float(2.0**34)  # mask additive bias magnitude (bf16-exact)
INV_SQRT_D = 1.0 / math.sqrt(D)

F32 = mybir.dt.float32
F32R = mybir.dt.float32r
BF16 = mybir.dt.bfloat16
EXP = mybir.ActivationFunctionType.Exp

# Tunables
# Colsum matmul dtype strategy:
#   "f32r_copy"   - extra DVE copies round (recip, p_un) to float32r (e8m11)
#                   so the colsum matmul streams at full rate; p_attn stays
#                   full fp32 precision.
#   "f32r_direct" - exp2 writes p_un as float32r directly (saves the copy,
#                   p_attn inherits ~2.4e-4 rounding).
#   "f32"         - plain fp32 matmul (4 cycles/row on the PE).
COLSUM_MODE = "f32r_copy"
E1_BF16 = False  # keep exp(QK) in fp32 (True trades ~1e-4 rel err for speed)


def build(nb=NB, sq=SQ, skv=SKV, d=D, repeat=1):
    """Build the single-core Bass program (SPMD across cores).

    repeat > 1 re-runs the whole body (same I/O) for benchmarking: the time
    difference between repeat=R and repeat=1 programs isolates steady-state
    kernel time from fixed NEFF/dispatch overheads."""
    assert d % P == 0 and sq % P == 0 and skv % 512 == 0
    nd = d // P  # contraction chunks (2)
    nqt = sq // P  # q tiles (8)
    nkv = skv // P  # kv 128-chunks (8)
    nvc = skv // 512  # kv 512-chunks (2)

    nc = bacc.Bacc("TRN2", target_bir_lowering=False, debug=False)
    qt_d = nc.dram_tensor("qt", [nb, d, sq], BF16, kind="ExternalInput")
    kt_d = nc.dram_tensor("kt", [nb, d, skv], BF16, kind="ExternalInput")
    v_d = nc.dram_tensor("v", [nb, skv, d], F32, kind="ExternalInput")
    mb_d = nc.dram_tensor("mb", [nb, sq, skv], BF16, kind="ExternalInput")
    pa_d = nc.dram_tensor("pa", [nb, sq, skv], F32, kind="ExternalOutput")
    o1_d = nc.dram_tensor("o1", [nb, d], F32, kind="ExternalOutput")

    e1_dt = BF16 if E1_BF16 else F32

    with tile.TileContext(nc) as tc:
        with (
            tc.tile_pool(name="qk_pool", bufs=2) as qk_pool,
            tc.tile_pool(name="v_pool", bufs=2) as v_pool,
            tc.tile_pool(name="mb_pool", bufs=3) as mb_pool,
            tc.tile_pool(name="e1_pool", bufs=2) as e1_pool,
            tc.tile_pool(name="t_pool", bufs=2) as t_pool,
            tc.tile_pool(name="pu_pool", bufs=2) as pu_pool,
            tc.tile_pool(name="pa_pool", bufs=3) as pa_pool,
            tc.tile_pool(name="small_pool", bufs=8) as small_pool,
            tc.tile_pool(name="c_pool", bufs=2) as c_pool,
            tc.tile_pool(name="ps_pool", bufs=3, space=bass.MemorySpace.PSUM) as ps_pool,
            tc.tile_pool(name="cs_pool", bufs=1, space=bass.MemorySpace.PSUM) as cs_pool,
            tc.tile_pool(name="f_pool", bufs=1, space=bass.MemorySpace.PSUM) as f_pool,
            tc.tile_pool(name="tp_pool", bufs=1, space=bass.MemorySpace.PSUM) as tp_pool,
        ):
            ones1 = c_pool.tile([1, 1], F32, tag="ones1")
            nc.vector.memset(ones1, 1.0)
            for b in [b for _ in range(repeat) for b in range(nb)]:
                # Q^T / K^T tiles: [d-chunk][128, seq] bf16, contiguous DMA.
                qts = []
                kts = []
                for dc in range(nd):
                    qtile = qk_pool.tile([P, sq], BF16, tag=f"qt{dc}")
                    nc.sync.dma_start(out=qtile, in_=qt_d[b, ts(dc, P), :])
                    qts.append(qtile)
                for dc in range(nd):
                    ktile = qk_pool.tile([P, skv], BF16, tag=f"kt{dc}")
                    nc.sync.dma_start(out=ktile, in_=kt_d[b, ts(dc, P), :])
                    kts.append(ktile)
                # V tiles: [kv-chunk][128, d] fp32, natural layout.
                vts = []
                for j in range(nkv):
                    vtile = v_pool.tile([P, d], F32, tag=f"v{j}")
                    nc.sync.dma_start(out=vtile, in_=v_d[b, ts(j, P), :])
                    vts.append(vtile)

                # colsum accumulator: c[0, kv] += recip.T @ p_unnorm
                cs = cs_pool.tile([1, skv], F32)

                for i in range(nqt):
                    mbt = mb_pool.tile([P, skv], BF16, tag="mbt")
                    nc.sync.dma_start(out=mbt, in_=mb_d[b, ts(i, P), :])

                    e1 = e1_pool.tile([P, skv], e1_dt, tag="e1")
                    for n in range(nvc):
                        ps = ps_pool.tile([P, 512], F32, tag="ps")
                        for dc in range(nd):
                            nc.tensor.matmul(
                                ps,
                                lhsT=qts[dc][:, ts(i, P)],
                                rhs=kts[dc][:, ts(n, 512)],
                                start=(dc == 0),
                                stop=(dc == nd - 1),
                            )
                        # e1 = exp(qk); scores scaling by 1/sqrt(d) is folded
                        # into the second exp below.
                        nc.scalar.activation(e1[:, ts(n, 512)], ps, EXP)

                    t = t_pool.tile([P, skv], e1_dt, tag="t")
                    nc.vector.tensor_add(t, e1, mbt)

                    pu_dt = F32R if COLSUM_MODE == "f32r_direct" else F32
                    pu = pu_pool.tile([P, skv], pu_dt, tag="pu")
                    z = small_pool.tile([P, 1], F32, tag="z")
                    nc.scalar.activation(
                        pu, t, EXP, scale=float(INV_SQRT_D), accum_out=z
                    )
                    rc = small_pool.tile([P, 1], F32, tag="rc")
                    nc.vector.reciprocal(rc, z)

                    pat = pa_pool.tile([P, skv], F32, tag="pat")
                    pu_f32 = pu.bitcast(F32) if COLSUM_MODE == "f32r_direct" else pu
                    nc.vector.tensor_scalar_mul(pat, pu_f32, rc)
                    nc.sync.dma_start(out=pa_d[b, ts(i, P), :], in_=pat)

                    # colsum of p_attn: out[1, kv] += sum_q recip[q]*pu[q, kv]
                    if COLSUM_MODE == "f32":
                        rc_mm, pu_mm = rc, pu
                    elif COLSUM_MODE == "f32r_direct":
                        rc_mm = small_pool.tile([P, 1], F32R, tag="rc_r")
                        nc.vector.tensor_copy(rc_mm, rc)
                        pu_mm = pu
                    else:  # f32r_copy
                        rc_mm = small_pool.tile([P, 1], F32R, tag="rc_r")
                        nc.vector.tensor_copy(rc_mm, rc)
                        pu_mm = pu_pool.tile([P, skv], F32R, tag="pu_r")
                        nc.vector.tensor_copy(pu_mm, pu)
                    for n in range(nvc):
                        nc.tensor.matmul(
                            cs[:, ts(n, 512)],
                            lhsT=rc_mm,
                            rhs=pu_mm[:, ts(n, 512)],
                            start=(i == 0),
                            stop=(i == nqt - 1),
                        )

                # Evacuate colsum, then transpose [1, skv] -> [128, nkv]
                # (ct[p, j] = c[j*128 + p]) via 8 tiny PE transposes.
                c_sb = c_pool.tile([1, skv], F32, tag="c_sb")
                nc.vector.tensor_copy(c_sb, cs)
                ct = c_pool.tile([P, nkv], F32, tag="ct")
                for j in range(nkv):
                    tp = tp_pool.tile([P, 1], F32, tag="tp")
                    nc.tensor.transpose(tp, c_sb[0:1, ts(j, P)], ones1)
                    nc.vector.tensor_copy(ct[:, ts(j, 1)], tp)

                # Final out = c.T @ V: small (8 matmuls of N=256), plain fp32.
                f_ps = f_pool.tile([1, d], F32)
                for j in range(nkv):
                    nc.tensor.matmul(
                        f_ps,
                        lhsT=ct[:, ts(j, 1)],
                        rhs=vts[j],
                        start=(j == 0),
                        stop=(j == nkv - 1),
                    )
                o_sb = c_pool.tile([1, d], F32, tag="o_sb")
                nc.scalar.copy(o_sb, f_ps)
                nc.sync.dma_start(out=o1_d[ts(b, 1), :], in_=o_sb)

    nc.compile()
    return nc


_CACHE = {}


def _built():
    if "nc" not in _CACHE:
        _CACHE["nc"] = build()
    return _CACHE["nc"]


def _prep_inputs(query, key, value, mask):
    """Host-side prep: cast/transpose Q,K to bf16 [B, D, S]; mask -> additive
    bf16 bias (mask-1)*2^34; shard over batch."""
    qT = np.ascontiguousarray(query.transpose(0, 2, 1)).astype(ml_dtypes.bfloat16)
    kT = np.ascontiguousarray(key.transpose(0, 2, 1)).astype(ml_dtypes.bfloat16)
    v = np.ascontiguousarray(value.astype(np.float32))
    mbias = ((mask != 0).astype(np.float32) - 1.0) * np.float32(LAM)
    mbias = mbias.astype(ml_dtypes.bfloat16)
    in_maps = []
    for c in range(NCORES):
        s = slice(c * NB, (c + 1) * NB)
        in_maps.append(
            {
                "qt": np.ascontiguousarray(qT[s]),
                "kt": np.ascontiguousarray(kT[s]),
                "v": v[s],
                "mb": np.ascontiguousarray(mbias[s]),
            }
        )
    return in_maps


def run(query, key, value, mask, **spmd_kwargs):
    """Run on 8 NeuronCores; returns (results, BassKernelResults)."""
    query = np.asarray(query, dtype=np.float32)
    key = np.asarray(key, dtype=np.float32)
    value = np.asarray(value, dtype=np.float32)
    mask = np.asarray(mask)
    nc = _built()
    in_maps = _prep_inputs(query, key, value, mask)
    res = run_bass_kernel_spmd(nc, in_maps, core_ids=list(range(NCORES)), **spmd_kwargs)
    out1 = np.concatenate([r["o1"] for r in res.results], axis=0).astype(np.float32)
    p_attn = np.concatenate([r["pa"] for r in res.results], axis=0).astype(np.float32)
    return (out1, p_attn), res


def kernel(query, key, value, mask):
    outs, _ = run(query, key, value, mask)
    return outs
